# revision 2
# baseline (speedup 1.0000x reference)
"""Trainium2 Bass kernel for causal multi-head attention (B=4, S=2048, E=1024, H=16).

Sharding: 8 cores = (batch b in 0..3) x (head-group g in 0..1); each core
computes one batch and 8 heads end-to-end:
  - column-parallel QKV projection (only its heads' columns)
  - causal attention for its 8 heads
  - row-parallel output projection -> partial [S, E]
The two partials per batch are summed on the host (plus b_proj). No on-device
collectives are needed.

Device dataflow (per core), all matmuls in bf16 with fp32 PSUM accumulation:
  - Q^T, K^T computed directly in [feature, token] layout (out = W^T @ x^T),
    so attention needs no transposes. K^T is pre-scaled by 1/sqrt(d)=0.125.
  - scores tile S^T[k,q] = (K^T tile).T @ Q^T ; exp via ACT; causal mask via
    a multiplicative 0/1 bf16 mask on diagonal tiles (fully-masked tiles are
    skipped entirely).
  - AV: lhsT = [V | ones] so the PSUM output rows 0..63 are O^T (unnormalized)
    and row 64 is the softmax denominator. Reciprocal + PE-broadcast + one
    DVE multiply produce normalized O^T in bf16.
  - c_proj contracts the stacked O^T [512, S] against W_proj rows.
"""

import os
import sys

import numpy as np

for _p in ("/opt/trn_rl_repo", "/root/.axon_site/_ro/trn_rl_repo"):
    if os.path.isdir(_p) and _p not in sys.path:
        sys.path.append(_p)

import ml_dtypes  # noqa: E402

import concourse.bass as bass  # noqa: E402
import concourse.tile as tile  # noqa: E402
from concourse import bacc, bass_utils, mybir  # noqa: E402

BF16 = ml_dtypes.bfloat16

B, S, E, H = 4, 2048, 1024, 16
D = E // H            # 64
NCORE = 8
HPC = H // 2          # heads per core = 8
KT = E // 128         # contraction tiles over E = 8
TB512 = S // 512      # 4
TB128 = S // 128      # 16
FPC = HPC * D         # features per core in attention output = 512

_cache: dict = {}


def _ts(i, n):
    return slice(i * n, (i + 1) * n)


def _build_program():
    bf = mybir.dt.bfloat16
    f32 = mybir.dt.float32
    nc = bacc.Bacc("TRN2", target_bir_lowering=False, debug=False)

    xt = nc.dram_tensor("xt", [128, KT, S], bf, kind="ExternalInput")
    wqk = nc.dram_tensor("wqk", [128, KT, 2 * FPC], bf, kind="ExternalInput")
    wv = nc.dram_tensor("wv", [128, KT, FPC], bf, kind="ExternalInput")
    wp = nc.dram_tensor("wp", [128, FPC // 128, E], bf, kind="ExternalInput")
    bqk = nc.dram_tensor("bqk", [1, 2 * FPC], bf, kind="ExternalInput")
    bv = nc.dram_tensor("bv", [1, FPC], bf, kind="ExternalInput")
    msk = nc.dram_tensor("msk", [128, 4, 512], bf, kind="ExternalInput")
    out = nc.dram_tensor("out", [S, E], f32, kind="ExternalOutput")

    Exp = mybir.ActivationFunctionType.Exp

    with tile.TileContext(nc) as tc:
        with (
            tc.tile_pool(name="big", bufs=1) as big,
            tc.tile_pool(name="mm", bufs=2, space="PSUM") as ps_mm,
            tc.tile_pool(name="sp", bufs=3, space="PSUM") as ps_sp,
            tc.tile_pool(name="av", bufs=2, space="PSUM") as ps_av,
            tc.tile_pool(name="bc", bufs=1, space="PSUM") as ps_bc,
            tc.tile_pool(name="pp", bufs=3) as pp,
            tc.tile_pool(name="sm", bufs=3) as sm,
            tc.tile_pool(name="ob", bufs=3) as ob,
        ):
            xt_sb = big.tile([128, KT, S], bf, tag="xt")
            wqk_sb = big.tile([128, KT, 2 * FPC], bf, tag="wqk")
            wv_sb = big.tile([128, KT, FPC], bf, tag="wv")
            wp_sb = big.tile([128, FPC // 128, E], bf, tag="wp")
            bqk_sb = big.tile([1, 2 * FPC], bf, tag="bqk")
            bv_sb = big.tile([1, FPC], bf, tag="bv")
            msk_sb = big.tile([128, 4, 512], bf, tag="msk")
            ones_sb = big.tile([1, S], bf, tag="ones")
            qT_sb = big.tile([128, 4, S], bf, tag="qT")
            kT_sb = big.tile([128, 4, S], bf, tag="kT")
            vone_sb = big.tile([128, TB128, HPC, D + 1], bf, tag="vone")
            oT_sb = big.tile([128, 4, S], bf, tag="oT")

            nc.sync.dma_start(out=wqk_sb, in_=wqk.ap())
            nc.sync.dma_start(out=xt_sb, in_=xt.ap())
            nc.sync.dma_start(out=bqk_sb, in_=bqk.ap())
            nc.sync.dma_start(out=wv_sb, in_=wv.ap())
            nc.sync.dma_start(out=bv_sb, in_=bv.ap())
            nc.sync.dma_start(out=msk_sb, in_=msk.ap())
            nc.sync.dma_start(out=wp_sb, in_=wp.ap())

            nc.vector.memset(ones_sb, 1.0)
            nc.vector.memset(vone_sb[:, :, :, D : D + 1], 1.0)

            # ---- Phase 1a: Q^T / K^T = W^T @ x^T (+bias), pair-stacked ----
            # fb 0..3 -> Q pairs, fb 4..7 -> K pairs. K^T pre-scaled by 0.125.
            for fb in range(8):
                for tb in range(TB512):
                    ps = ps_mm.tile([128, 512], f32, tag="mmps")
                    for kt in range(KT):
                        nc.tensor.matmul(
                            ps,
                            lhsT=wqk_sb[:, kt, _ts(fb, 128)],
                            rhs=xt_sb[:, kt, _ts(tb, 512)],
                            start=(kt == 0),
                            stop=False,
                        )
                    nc.tensor.matmul(
                        ps,
                        lhsT=bqk_sb[0:1, _ts(fb, 128)],
                        rhs=ones_sb[0:1, _ts(tb, 512)],
                        start=False,
                        stop=True,
                    )
                    if fb < 4:
                        nc.scalar.copy(qT_sb[:, fb, _ts(tb, 512)], ps)
                    else:
                        nc.scalar.mul(kT_sb[:, fb - 4, _ts(tb, 512)], ps, 0.125)

            # ---- Phase 1b: V = x @ Wv (+bias), [token, feature] layout ----
            for tb in range(TB128):
                ps = ps_mm.tile([128, 512], f32, tag="mmps")
                for kt in range(KT):
                    nc.tensor.matmul(
                        ps,
                        lhsT=xt_sb[:, kt, _ts(tb, 128)],
                        rhs=wv_sb[:, kt, :],
                        start=(kt == 0),
                        stop=False,
                    )
                nc.tensor.matmul(
                    ps,
                    lhsT=ones_sb[0:1, 0:128],
                    rhs=bv_sb[0:1, :],
                    start=False,
                    stop=True,
                )
                nc.scalar.copy(
                    vone_sb[:, tb, :, 0:D],
                    ps[:, :].rearrange("p (h d) -> p h d", d=D),
                )

            # ---- Phase 2: causal attention per head ----
            for h in range(HPC):
                pair = h // 2
                qoff = (h % 2) * D
                for qb in range(TB512):
                    nkt = 4 * qb + 4  # causal: k-tiles 0 .. 4qb+3
                    av = ps_av.tile([D + 1, 512], f32, tag="av")
                    pend = None

                    def do_av(ki, p_sb, av=av, h=h, nkt=nkt):
                        nc.tensor.matmul(
                            av,
                            lhsT=vone_sb[:, ki, h, :],
                            rhs=p_sb,
                            start=(ki == 0),
                            stop=(ki == nkt - 1),
                        )

                    for ki in range(nkt):
                        sp = ps_sp.tile([128, 512], f32, tag="sp")
                        nc.tensor.matmul(
                            sp,
                            lhsT=kT_sb[qoff : qoff + D, pair, _ts(ki, 128)],
                            rhs=qT_sb[qoff : qoff + D, pair, _ts(qb, 512)],
                            start=True,
                            stop=True,
                        )
                        if pend is not None:
                            do_av(*pend)
                        p_sb = pp.tile([128, 512], bf, tag="p")
                        nc.scalar.activation(p_sb, sp, Exp)
                        if ki >= 4 * qb:
                            nc.vector.tensor_mul(
                                p_sb, p_sb, msk_sb[:, ki - 4 * qb, :]
                            )
                        pend = (ki, p_sb)
                    do_av(*pend)

                    # normalize: O^T[0:D] * (1/denom) broadcast over partitions.
                    # All PSUM reads go through ACT (DVE reads of PE-written
                    # PSUM race the matmul drain on HW); DVE stays SBUF-only.
                    d_sb = sm.tile([1, 512], f32, tag="d")
                    nc.scalar.copy(d_sb, av[D : D + 1, :])
                    o_sb = sm.tile([D, 512], f32, tag="o")
                    nc.scalar.copy(o_sb, av[0:D, :])
                    r_sb = sm.tile([1, 512], bf, tag="r")
                    with nc.allow_low_precision(reason="softmax recip bf16"):
                        nc.vector.reciprocal(r_sb, d_sb)
                    bc = ps_bc.tile([D, 512], f32, tag="bc")
                    nc.tensor.matmul(
                        bc, lhsT=ones_sb[0:1, 0:D], rhs=r_sb, start=True, stop=True
                    )
                    bc_sb = sm.tile([D, 512], f32, tag="bcsb")
                    nc.scalar.copy(bc_sb, bc)
                    nc.vector.tensor_mul(
                        oT_sb[qoff : qoff + D, pair, _ts(qb, 512)],
                        o_sb,
                        bc_sb,
                    )

            # ---- Phase 3: c_proj partial = O^T.T @ Wp ----
            for tb in range(TB128):
                for eb in range(2):
                    ps = ps_mm.tile([128, 512], f32, tag="mmps")
                    for fg in range(FPC // 128):
                        nc.tensor.matmul(
                            ps,
                            lhsT=oT_sb[:, fg, _ts(tb, 128)],
                            rhs=wp_sb[:, fg, _ts(eb, 512)],
                            start=(fg == 0),
                            stop=(fg == FPC // 128 - 1),
                        )
                    o_sb = ob.tile([128, 512], f32, tag="osb")
                    nc.scalar.copy(o_sb, ps)
                    nc.sync.dma_start(
                        out=out.ap()[_ts(tb, 128), _ts(eb, 512)], in_=o_sb
                    )

    nc.compile()
    return nc


def _part_major(a, p=128):
    """[n*128, m] -> [128, n, m] with partition index innermost in rows."""
    n = a.shape[0] // p
    return np.ascontiguousarray(a.reshape(n, p, a.shape[1]).transpose(1, 0, 2))


def make_in_maps(x, W_attn, b_attn, W_proj):
    """Build the 8 per-core input maps (core = 2*b + g)."""
    x = np.asarray(x, dtype=np.float32)
    W_attn = np.asarray(W_attn, dtype=np.float32)
    b_attn = np.asarray(b_attn, dtype=np.float32)
    W_proj = np.asarray(W_proj, dtype=np.float32)

    # causal 0/1 masks for the 4 diagonal alignments (k-tile 128 vs q-block 512)
    kk = np.arange(128)[:, None]
    qq = np.arange(512)[None, :]
    msk = np.stack(
        [(qq >= j * 128 + kk) for j in range(4)], axis=1
    ).astype(BF16)  # [128, 4, 512]

    in_maps = []
    for b in range(B):
        xt = _part_major(np.ascontiguousarray(x[b].T)).astype(BF16)  # [128,8,S]
        for g in range(2):
            qs = W_attn[:, g * FPC : (g + 1) * FPC]
            ks = W_attn[:, E + g * FPC : E + (g + 1) * FPC]
            vs = W_attn[:, 2 * E + g * FPC : 2 * E + (g + 1) * FPC]
            wqk = _part_major(np.concatenate([qs, ks], axis=1)).astype(BF16)
            wv = _part_major(vs).astype(BF16)
            wp = _part_major(W_proj[g * FPC : (g + 1) * FPC, :]).astype(BF16)
            bq = b_attn[g * FPC : (g + 1) * FPC]
            bk = b_attn[E + g * FPC : E + (g + 1) * FPC]
            bqk = np.concatenate([bq, bk])[None, :].astype(BF16)
            bv = b_attn[2 * E + g * FPC : 2 * E + (g + 1) * FPC][None, :].astype(
                BF16
            )
            in_maps.append(
                {
                    "xt": xt,
                    "wqk": np.ascontiguousarray(wqk),
                    "wv": np.ascontiguousarray(wv),
                    "wp": np.ascontiguousarray(wp),
                    "bqk": np.ascontiguousarray(bqk),
                    "bv": np.ascontiguousarray(bv),
                    "msk": np.ascontiguousarray(msk),
                }
            )
    return in_maps


def get_program():
    if "nc" not in _cache:
        _cache["nc"] = _build_program()
    return _cache["nc"]


def gather(results, b_proj):
    b_proj = np.asarray(b_proj, dtype=np.float32)
    out = np.empty((B, S, E), dtype=np.float32)
    for b in range(B):
        out[b] = results[2 * b]["out"] + results[2 * b + 1]["out"] + b_proj
    return out


def kernel(x, W_attn, b_attn, W_proj, b_proj):
    nc = get_program()
    in_maps = make_in_maps(x, W_attn, b_attn, W_proj)
    res = bass_utils.run_bass_kernel_spmd(nc, in_maps, core_ids=list(range(NCORE)))
    return gather(res.results, b_proj)


# revision 4
# speedup vs baseline: 1.0515x; 1.0515x over previous
"""Trainium2 Bass kernel for causal multi-head attention (B=4, S=2048, E=1024, H=16).

Sharding: 8 cores = (batch b in 0..3) x (head-group g in 0..1); each core
computes one batch and 8 heads end-to-end:
  - column-parallel QKV projection (only its heads' columns)
  - causal attention for its 8 heads
  - row-parallel output projection -> partial [S, E]
The two partials per batch are summed on the host (plus b_proj). No on-device
collectives are needed.

Device dataflow (per core), all matmuls in bf16 with fp32 PSUM accumulation:
  - Q^T, K^T computed directly in [feature, token] layout (out = W^T @ x^T),
    so attention needs no transposes. K^T is pre-scaled by 1/sqrt(d)=0.125.
  - scores S^T[k,q] = (K^T tile).T @ Q^T, two k-tiles fused per [128,1024]
    PSUM tile; one ACT exp evacuates both; causal mask via multiplicative
    0/1 bf16 masks on diagonal tiles (fully-masked tiles skipped).
  - AV: lhsT = [V | ones]; PSUM rows 0..63 = O^T (unnormalized), row 64 =
    softmax denominator. 1/denom via ACT exp(-ln(d)) (DVE reciprocal is
    ~3.3us for a 1-partition row; ACT is ~0.9us), PE ones-matmul broadcast,
    one DVE multiply -> normalized O^T bf16. Normalize chains are deferred
    into the next (head, q-block) group so the PE never stalls on them.
  - All PSUM evacuation goes through ACT: DVE reads of PE-written PSUM race
    the matmul drain on HW (observed flaky garbage on first execution).
  - c_proj contracts the stacked O^T [512, S] against W_proj rows.
"""

import os
import sys

import numpy as np

for _p in ("/opt/trn_rl_repo", "/root/.axon_site/_ro/trn_rl_repo"):
    if os.path.isdir(_p) and _p not in sys.path:
        sys.path.append(_p)

import ml_dtypes  # noqa: E402

import concourse.bass as bass  # noqa: E402
import concourse.tile as tile  # noqa: E402
from concourse import bacc, bass_utils, mybir  # noqa: E402

BF16 = ml_dtypes.bfloat16

B, S, E, H = 4, 2048, 1024, 16
D = E // H            # 64
NCORE = 8
HPC = H // 2          # heads per core = 8
KT = E // 128         # contraction tiles over E = 8
TB512 = S // 512      # 4
TB128 = S // 128      # 16
FPC = HPC * D         # features per core in attention output = 512

_cache: dict = {}


def _ts(i, n):
    return slice(i * n, (i + 1) * n)


def _build_program():
    bf = mybir.dt.bfloat16
    f32 = mybir.dt.float32
    nc = bacc.Bacc("TRN2", target_bir_lowering=False, debug=False)

    xt = nc.dram_tensor("xt", [128, KT, S], bf, kind="ExternalInput")
    wqk = nc.dram_tensor("wqk", [128, KT, 2 * FPC], bf, kind="ExternalInput")
    wv = nc.dram_tensor("wv", [128, KT, FPC], bf, kind="ExternalInput")
    wp = nc.dram_tensor("wp", [128, FPC // 128, E], bf, kind="ExternalInput")
    bqk = nc.dram_tensor("bqk", [1, 2 * FPC], bf, kind="ExternalInput")
    bv = nc.dram_tensor("bv", [1, FPC], bf, kind="ExternalInput")
    msk = nc.dram_tensor("msk", [128, 4, 512], bf, kind="ExternalInput")
    out = nc.dram_tensor("out", [S, E], f32, kind="ExternalOutput")

    Exp = mybir.ActivationFunctionType.Exp
    Ln = mybir.ActivationFunctionType.Ln

    with tile.TileContext(nc) as tc:
        with (
            tc.tile_pool(name="big", bufs=1) as big,
            tc.tile_pool(name="pp", bufs=3) as pp,
            tc.tile_pool(name="sm", bufs=3) as sm,
            tc.tile_pool(name="ob", bufs=3) as ob,
        ):
            xt_sb = big.tile([128, KT, S], bf, tag="xt")
            wqk_sb = big.tile([128, KT, 2 * FPC], bf, tag="wqk")
            wv_sb = big.tile([128, KT, FPC], bf, tag="wv")
            wp_sb = big.tile([128, FPC // 128, E], bf, tag="wp")
            bqk_sb = big.tile([1, 2 * FPC], bf, tag="bqk")
            bv_sb = big.tile([1, FPC], bf, tag="bv")
            msk_sb = big.tile([128, 4, 512], bf, tag="msk")
            ones_sb = big.tile([1, S], bf, tag="ones")
            qT_sb = big.tile([128, 4, S], bf, tag="qT")
            kT_sb = big.tile([128, 4, S], bf, tag="kT")
            vone_sb = big.tile([128, TB128, HPC, D + 1], bf, tag="vone")
            oT_sb = big.tile([128, 4, S], bf, tag="oT")

            nc.sync.dma_start(out=wqk_sb, in_=wqk.ap())
            nc.sync.dma_start(out=xt_sb, in_=xt.ap())
            nc.sync.dma_start(out=bqk_sb, in_=bqk.ap())
            nc.sync.dma_start(out=wv_sb, in_=wv.ap())
            nc.sync.dma_start(out=bv_sb, in_=bv.ap())
            nc.sync.dma_start(out=msk_sb, in_=msk.ap())
            nc.sync.dma_start(out=wp_sb, in_=wp.ap())

            nc.vector.memset(ones_sb, 1.0)
            nc.vector.memset(vone_sb[:, :, :, D : D + 1], 1.0)

            # ---- Phase 1a: Q^T / K^T = W^T @ x^T (+bias), pair-stacked ----
            # fb 0..3 -> Q pairs, fb 4..7 -> K pairs. K^T pre-scaled by 0.125.
            with tc.tile_pool(name="ps1", bufs=3, space="PSUM") as ps1:
                for fb in range(8):
                    for tbp in range(TB512 // 2):
                        ps = ps1.tile([128, 1024], f32, tag="qkv")
                        for half in range(2):
                            tb = 2 * tbp + half
                            hs = _ts(half, 512)
                            for kt in range(KT):
                                nc.tensor.matmul(
                                    ps[:, hs],
                                    lhsT=wqk_sb[:, kt, _ts(fb, 128)],
                                    rhs=xt_sb[:, kt, _ts(tb, 512)],
                                    start=(kt == 0),
                                    stop=False,
                                )
                            nc.tensor.matmul(
                                ps[:, hs],
                                lhsT=bqk_sb[0:1, _ts(fb, 128)],
                                rhs=ones_sb[0:1, _ts(tb, 512)],
                                start=False,
                                stop=True,
                            )
                        if fb < 4:
                            nc.scalar.copy(qT_sb[:, fb, _ts(tbp, 1024)], ps)
                        else:
                            nc.scalar.mul(kT_sb[:, fb - 4, _ts(tbp, 1024)], ps, 0.125)

                # ---- Phase 1b: V = x @ Wv (+bias), [token, feature] layout --
                for tbp in range(TB128 // 2):
                    ps = ps1.tile([128, 1024], f32, tag="qkv")
                    for half in range(2):
                        tb = 2 * tbp + half
                        hs = _ts(half, 512)
                        for kt in range(KT):
                            nc.tensor.matmul(
                                ps[:, hs],
                                lhsT=xt_sb[:, kt, _ts(tb, 128)],
                                rhs=wv_sb[:, kt, :],
                                start=(kt == 0),
                                stop=False,
                            )
                        nc.tensor.matmul(
                            ps[:, hs],
                            lhsT=ones_sb[0:1, 0:128],
                            rhs=bv_sb[0:1, :],
                            start=False,
                            stop=True,
                        )
                    nc.scalar.copy(
                        vone_sb[:, _ts(tbp, 2), :, 0:D],
                        ps[:, :].rearrange("p (t h d) -> p t h d", t=2, d=D),
                    )

            # ---- Phase 2: causal attention per head ----
            with (
                tc.tile_pool(name="sp", bufs=2, space="PSUM") as ps_sp,
                tc.tile_pool(name="av", bufs=3, space="PSUM") as ps_av,
                tc.tile_pool(name="bc", bufs=1, space="PSUM") as ps_bc,
            ):
                deferred = []  # pending normalize chains (one per group)

                def normalize(av, h, pair, qoff, qb):
                    t1 = sm.tile([1, 512], f32, tag="t1")
                    nc.scalar.activation(t1, av[D : D + 1, :], Ln)
                    r_sb = sm.tile([1, 512], bf, tag="r")
                    nc.scalar.activation(r_sb, t1, Exp, scale=-1.0)
                    o_all = sm.tile([D, 512], f32, tag="o")
                    nc.scalar.copy(o_all, av[0:D, :])
                    bc = ps_bc.tile([D, 512], f32, tag="bc")
                    nc.tensor.matmul(
                        bc, lhsT=ones_sb[0:1, 0:D], rhs=r_sb, start=True, stop=True
                    )
                    bc_sb = sm.tile([D, 512], f32, tag="bcsb")
                    nc.scalar.copy(bc_sb, bc)
                    nc.vector.tensor_mul(
                        oT_sb[qoff : qoff + D, pair, _ts(qb, 512)], o_all, bc_sb
                    )

                for h in range(HPC):
                    pair = h // 2
                    qoff = (h % 2) * D
                    for qb in range(TB512):
                        npairs = 2 * qb + 2  # fused ki-pairs (4qb+4 k-tiles)
                        av = ps_av.tile([D + 1, 512], f32, tag="av")
                        pend = None

                        def do_av(p, p_sb, av=av, h=h, npairs=npairs):
                            for half in range(2):
                                ki = 2 * p + half
                                nc.tensor.matmul(
                                    av,
                                    lhsT=vone_sb[:, ki, h, :],
                                    rhs=p_sb[:, _ts(half, 512)],
                                    start=(ki == 0),
                                    stop=(ki == 2 * npairs - 1),
                                )

                        for p in range(npairs):
                            sp = ps_sp.tile([128, 1024], f32, tag="sp")
                            for half in range(2):
                                ki = 2 * p + half
                                nc.tensor.matmul(
                                    sp[:, _ts(half, 512)],
                                    lhsT=kT_sb[qoff : qoff + D, pair, _ts(ki, 128)],
                                    rhs=qT_sb[qoff : qoff + D, pair, _ts(qb, 512)],
                                    start=True,
                                    stop=True,
                                )
                            if pend is not None:
                                do_av(*pend)
                            p_sb = pp.tile([128, 1024], bf, tag="p")
                            nc.scalar.activation(p_sb, sp, Exp)
                            if p >= 2 * qb:  # both halves are diagonal tiles
                                j = 2 * (p - 2 * qb)
                                nc.vector.tensor_mul(
                                    p_sb,
                                    p_sb,
                                    msk_sb[:, j : j + 2, :].rearrange(
                                        "k j q -> k (j q)"
                                    ),
                                )
                            pend = (p, p_sb)
                            if p == 1:
                                while deferred:
                                    deferred.pop(0)()
                        do_av(*pend)
                        deferred.append(
                            lambda av=av, h=h, pair=pair, qoff=qoff, qb=qb: normalize(
                                av, h, pair, qoff, qb
                            )
                        )
                while deferred:
                    deferred.pop(0)()

            # ---- Phase 3: c_proj partial = O^T.T @ Wp ----
            with tc.tile_pool(name="ps3", bufs=3, space="PSUM") as ps3:
                for tb in range(TB128):
                    ps = ps3.tile([128, 1024], f32, tag="proj")
                    for eb in range(2):
                        for fg in range(FPC // 128):
                            nc.tensor.matmul(
                                ps[:, _ts(eb, 512)],
                                lhsT=oT_sb[:, fg, _ts(tb, 128)],
                                rhs=wp_sb[:, fg, _ts(eb, 512)],
                                start=(fg == 0),
                                stop=(fg == FPC // 128 - 1),
                            )
                    o_sb = ob.tile([128, 1024], f32, tag="osb")
                    nc.scalar.copy(o_sb, ps)
                    nc.sync.dma_start(out=out.ap()[_ts(tb, 128), :], in_=o_sb)

    nc.compile()
    return nc


def _part_major(a, p=128):
    """[n*128, m] -> [128, n, m] with partition index innermost in rows."""
    n = a.shape[0] // p
    return np.ascontiguousarray(a.reshape(n, p, a.shape[1]).transpose(1, 0, 2))


def make_in_maps(x, W_attn, b_attn, W_proj):
    """Build the 8 per-core input maps (core = 2*b + g)."""
    x = np.asarray(x, dtype=np.float32)
    W_attn = np.asarray(W_attn, dtype=np.float32)
    b_attn = np.asarray(b_attn, dtype=np.float32)
    W_proj = np.asarray(W_proj, dtype=np.float32)

    # causal 0/1 masks for the 4 diagonal alignments (k-tile 128 vs q-block 512)
    kk = np.arange(128)[:, None]
    qq = np.arange(512)[None, :]
    msk = np.stack(
        [(qq >= j * 128 + kk) for j in range(4)], axis=1
    ).astype(BF16)  # [128, 4, 512]

    in_maps = []
    for b in range(B):
        xt = _part_major(np.ascontiguousarray(x[b].T)).astype(BF16)  # [128,8,S]
        for g in range(2):
            qs = W_attn[:, g * FPC : (g + 1) * FPC]
            ks = W_attn[:, E + g * FPC : E + (g + 1) * FPC]
            vs = W_attn[:, 2 * E + g * FPC : 2 * E + (g + 1) * FPC]
            wqk = _part_major(np.concatenate([qs, ks], axis=1)).astype(BF16)
            wv = _part_major(vs).astype(BF16)
            wp = _part_major(W_proj[g * FPC : (g + 1) * FPC, :]).astype(BF16)
            bq = b_attn[g * FPC : (g + 1) * FPC]
            bk = b_attn[E + g * FPC : E + (g + 1) * FPC]
            bqk = np.concatenate([bq, bk])[None, :].astype(BF16)
            bv = b_attn[2 * E + g * FPC : 2 * E + (g + 1) * FPC][None, :].astype(
                BF16
            )
            in_maps.append(
                {
                    "xt": xt,
                    "wqk": np.ascontiguousarray(wqk),
                    "wv": np.ascontiguousarray(wv),
                    "wp": np.ascontiguousarray(wp),
                    "bqk": np.ascontiguousarray(bqk),
                    "bv": np.ascontiguousarray(bv),
                    "msk": np.ascontiguousarray(msk),
                }
            )
    return in_maps


def get_program():
    if "nc" not in _cache:
        _cache["nc"] = _build_program()
    return _cache["nc"]


def gather(results, b_proj):
    b_proj = np.asarray(b_proj, dtype=np.float32)
    out = np.empty((B, S, E), dtype=np.float32)
    for b in range(B):
        out[b] = results[2 * b]["out"] + results[2 * b + 1]["out"] + b_proj
    return out


def kernel(x, W_attn, b_attn, W_proj, b_proj):
    nc = get_program()
    in_maps = make_in_maps(x, W_attn, b_attn, W_proj)
    res = bass_utils.run_bass_kernel_spmd(nc, in_maps, core_ids=list(range(NCORE)))
    return gather(res.results, b_proj)


# revision 5
# speedup vs baseline: 1.1991x; 1.1403x over previous
"""Trainium2 Bass kernel for causal multi-head attention (B=4, S=2048, E=1024, H=16).

Sharding: 8 cores = (batch b in 0..3) x (head-group g in 0..1); each core
computes one batch and 8 heads end-to-end:
  - column-parallel QKV projection (only its heads' columns)
  - causal attention for its 8 heads
  - row-parallel output projection -> partial [S, E]
The two partials per batch are summed on the host (plus b_proj). No on-device
collectives are needed.

Device dataflow (per core), all matmuls in bf16 with fp32 PSUM accumulation:
  - Q^T, K^T computed directly in [feature, token] layout (out = W^T @ x^T),
    so attention needs no transposes. K^T is pre-scaled by 1/sqrt(d)=0.125.
  - scores S^T[k,q] = (K^T tile).T @ Q^T, two k-tiles fused per [128,1024]
    PSUM tile; one ACT exp evacuates both; causal mask via multiplicative
    0/1 bf16 masks on diagonal tiles (fully-masked tiles skipped).
  - AV: lhsT = [V | ones]; PSUM rows 0..63 = O^T (unnormalized), row 64 =
    softmax denominator. 1/denom via ACT exp(-ln(d)) (DVE reciprocal is
    ~3.3us for a 1-partition row; ACT is ~0.9us), PE ones-matmul broadcast,
    one DVE multiply -> normalized O^T bf16. Normalize chains are deferred
    into the next (head, q-block) group so the PE never stalls on them.
  - All PSUM evacuation goes through ACT: DVE reads of PE-written PSUM race
    the matmul drain on HW (observed flaky garbage on first execution).
  - c_proj contracts the stacked O^T [512, S] against W_proj rows.
"""

import os
import sys

import numpy as np

for _p in ("/opt/trn_rl_repo", "/root/.axon_site/_ro/trn_rl_repo"):
    if os.path.isdir(_p) and _p not in sys.path:
        sys.path.append(_p)

import ml_dtypes  # noqa: E402

import concourse.bass as bass  # noqa: E402
import concourse.tile as tile  # noqa: E402
from concourse import bacc, bass_utils, hw_specs, mybir  # noqa: E402

# The act-table chooser assigns each activation the first table set containing
# its function, which ping-pongs Exp ("exp_and_others") and Ln ("natural_log")
# and inserts a 1.3us ACT_TABLE_LOAD per normalize chain (~65 per program).
# Restrict Exp/Ln to the combined "natural_log_exp_and_others" set (its
# act_func_set_id is preserved) so one load covers the whole kernel.
_orig_gat = hw_specs.get_activation_tables


def _shaped_gat(arch):
    t = _orig_gat(arch)
    if "natural_log_exp_and_others" in t:
        for name, funcs in t.items():
            if name != "natural_log_exp_and_others":
                funcs.discard(mybir.ActivationFunctionType.Exp)
                funcs.discard(mybir.ActivationFunctionType.Ln)
    return t


hw_specs.get_activation_tables = _shaped_gat
bacc.get_activation_tables = _shaped_gat

BF16 = ml_dtypes.bfloat16

B, S, E, H = 4, 2048, 1024, 16
D = E // H            # 64
NCORE = 8
HPC = H // 2          # heads per core = 8
KT = E // 128         # contraction tiles over E = 8
TB512 = S // 512      # 4
TB128 = S // 128      # 16
FPC = HPC * D         # features per core in attention output = 512

_cache: dict = {}


def _ts(i, n):
    return slice(i * n, (i + 1) * n)


def _build_program():
    bf = mybir.dt.bfloat16
    f32 = mybir.dt.float32
    nc = bacc.Bacc("TRN2", target_bir_lowering=False, debug=False)

    xt = nc.dram_tensor("xt", [128, KT, S], bf, kind="ExternalInput")
    wqk = nc.dram_tensor("wqk", [128, KT, 2 * FPC], bf, kind="ExternalInput")
    wv = nc.dram_tensor("wv", [128, KT, FPC], bf, kind="ExternalInput")
    wp = nc.dram_tensor("wp", [128, FPC // 128, E], bf, kind="ExternalInput")
    bqk = nc.dram_tensor("bqk", [1, 2 * FPC], bf, kind="ExternalInput")
    bv = nc.dram_tensor("bv", [1, FPC], bf, kind="ExternalInput")
    msk = nc.dram_tensor("msk", [128, 4, 512], bf, kind="ExternalInput")
    out = nc.dram_tensor("out", [S, E], f32, kind="ExternalOutput")

    Exp = mybir.ActivationFunctionType.Exp
    Ln = mybir.ActivationFunctionType.Ln

    with tile.TileContext(nc) as tc:
        with (
            tc.tile_pool(name="big", bufs=1) as big,
            tc.tile_pool(name="pp", bufs=3) as pp,
            tc.tile_pool(name="sm", bufs=3) as sm,
            tc.tile_pool(name="ob", bufs=3) as ob,
        ):
            xt_sb = big.tile([128, KT, S], bf, tag="xt")
            wqk_sb = big.tile([128, KT, 2 * FPC], bf, tag="wqk")
            wv_sb = big.tile([128, KT, FPC], bf, tag="wv")
            wp_sb = big.tile([128, FPC // 128, E], bf, tag="wp")
            bqk_sb = big.tile([1, 2 * FPC], bf, tag="bqk")
            bv_sb = big.tile([1, FPC], bf, tag="bv")
            msk_sb = big.tile([128, 4, 512], bf, tag="msk")
            ones_sb = big.tile([1, S], bf, tag="ones")
            qT_sb = big.tile([128, 4, S], bf, tag="qT")
            kT_sb = big.tile([128, 4, S], bf, tag="kT")
            vone_sb = big.tile([128, TB128, HPC, D + 1], bf, tag="vone")
            oT_sb = big.tile([128, 4, S], bf, tag="oT")

            nc.sync.dma_start(out=wqk_sb, in_=wqk.ap())
            nc.sync.dma_start(out=xt_sb, in_=xt.ap())
            nc.sync.dma_start(out=bqk_sb, in_=bqk.ap())
            nc.sync.dma_start(out=wv_sb, in_=wv.ap())
            nc.sync.dma_start(out=bv_sb, in_=bv.ap())
            nc.sync.dma_start(out=msk_sb, in_=msk.ap())
            nc.sync.dma_start(out=wp_sb, in_=wp.ap())

            nc.vector.memset(ones_sb, 1.0)
            nc.vector.memset(vone_sb[:, :, :, D : D + 1], 1.0)

            # ---- Phase 1a: Q^T / K^T = W^T @ x^T (+bias), pair-stacked ----
            # fb 0..3 -> Q pairs, fb 4..7 -> K pairs. K^T pre-scaled by 0.125.
            with tc.tile_pool(name="ps1", bufs=3, space="PSUM") as ps1:
                for fb in range(8):
                    for tbp in range(TB512 // 2):
                        ps = ps1.tile([128, 1024], f32, tag="qkv")
                        for half in range(2):
                            tb = 2 * tbp + half
                            hs = _ts(half, 512)
                            for kt in range(KT):
                                nc.tensor.matmul(
                                    ps[:, hs],
                                    lhsT=wqk_sb[:, kt, _ts(fb, 128)],
                                    rhs=xt_sb[:, kt, _ts(tb, 512)],
                                    start=(kt == 0),
                                    stop=False,
                                )
                            nc.tensor.matmul(
                                ps[:, hs],
                                lhsT=bqk_sb[0:1, _ts(fb, 128)],
                                rhs=ones_sb[0:1, _ts(tb, 512)],
                                start=False,
                                stop=True,
                            )
                        if fb < 4:
                            nc.scalar.copy(qT_sb[:, fb, _ts(tbp, 1024)], ps)
                        else:
                            nc.scalar.mul(kT_sb[:, fb - 4, _ts(tbp, 1024)], ps, 0.125)

                # ---- Phase 1b: V = x @ Wv (+bias), [token, feature] layout --
                for tbp in range(TB128 // 2):
                    ps = ps1.tile([128, 1024], f32, tag="qkv")
                    for half in range(2):
                        tb = 2 * tbp + half
                        hs = _ts(half, 512)
                        for kt in range(KT):
                            nc.tensor.matmul(
                                ps[:, hs],
                                lhsT=xt_sb[:, kt, _ts(tb, 128)],
                                rhs=wv_sb[:, kt, :],
                                start=(kt == 0),
                                stop=False,
                            )
                        nc.tensor.matmul(
                            ps[:, hs],
                            lhsT=ones_sb[0:1, 0:128],
                            rhs=bv_sb[0:1, :],
                            start=False,
                            stop=True,
                        )
                    nc.scalar.copy(
                        vone_sb[:, _ts(tbp, 2), :, 0:D],
                        ps[:, :].rearrange("p (t h d) -> p t h d", t=2, d=D),
                    )

            # ---- Phase 2: causal attention per head ----
            with (
                tc.tile_pool(name="sp", bufs=2, space="PSUM") as ps_sp,
                tc.tile_pool(name="av", bufs=3, space="PSUM") as ps_av,
                tc.tile_pool(name="bc", bufs=1, space="PSUM") as ps_bc,
            ):
                deferred = []  # pending normalize chains (one per group)

                def normalize(av, h, pair, qoff, qb):
                    t1 = sm.tile([1, 512], f32, tag="t1")
                    nc.scalar.activation(t1, av[D : D + 1, :], Ln)
                    r_sb = sm.tile([1, 512], bf, tag="r")
                    nc.scalar.activation(r_sb, t1, Exp, scale=-1.0)
                    o_all = sm.tile([D, 512], f32, tag="o")
                    nc.scalar.copy(o_all, av[0:D, :])
                    bc = ps_bc.tile([D, 512], f32, tag="bc")
                    nc.tensor.matmul(
                        bc, lhsT=ones_sb[0:1, 0:D], rhs=r_sb, start=True, stop=True
                    )
                    bc_sb = sm.tile([D, 512], f32, tag="bcsb")
                    nc.scalar.copy(bc_sb, bc)
                    nc.vector.tensor_mul(
                        oT_sb[qoff : qoff + D, pair, _ts(qb, 512)], o_all, bc_sb
                    )

                for h in range(HPC):
                    pair = h // 2
                    qoff = (h % 2) * D
                    for qb in range(TB512):
                        npairs = 2 * qb + 2  # fused ki-pairs (4qb+4 k-tiles)
                        av = ps_av.tile([D + 1, 512], f32, tag="av")
                        pend = None

                        def do_av(p, p_sb, av=av, h=h, npairs=npairs):
                            for half in range(2):
                                ki = 2 * p + half
                                nc.tensor.matmul(
                                    av,
                                    lhsT=vone_sb[:, ki, h, :],
                                    rhs=p_sb[:, _ts(half, 512)],
                                    start=(ki == 0),
                                    stop=(ki == 2 * npairs - 1),
                                )

                        for p in range(npairs):
                            sp = ps_sp.tile([128, 1024], f32, tag="sp")
                            for half in range(2):
                                ki = 2 * p + half
                                nc.tensor.matmul(
                                    sp[:, _ts(half, 512)],
                                    lhsT=kT_sb[qoff : qoff + D, pair, _ts(ki, 128)],
                                    rhs=qT_sb[qoff : qoff + D, pair, _ts(qb, 512)],
                                    start=True,
                                    stop=True,
                                )
                            if pend is not None:
                                do_av(*pend)
                            p_sb = pp.tile([128, 1024], bf, tag="p")
                            nc.scalar.activation(p_sb, sp, Exp)
                            if p >= 2 * qb:  # both halves are diagonal tiles
                                j = 2 * (p - 2 * qb)
                                nc.vector.tensor_mul(
                                    p_sb,
                                    p_sb,
                                    msk_sb[:, j : j + 2, :].rearrange(
                                        "k j q -> k (j q)"
                                    ),
                                )
                            pend = (p, p_sb)
                            if p == 1:
                                while deferred:
                                    deferred.pop(0)()
                        do_av(*pend)
                        deferred.append(
                            lambda av=av, h=h, pair=pair, qoff=qoff, qb=qb: normalize(
                                av, h, pair, qoff, qb
                            )
                        )
                while deferred:
                    deferred.pop(0)()

            # ---- Phase 3: c_proj partial = O^T.T @ Wp ----
            with tc.tile_pool(name="ps3", bufs=3, space="PSUM") as ps3:
                for tb in range(TB128):
                    ps = ps3.tile([128, 1024], f32, tag="proj")
                    for eb in range(2):
                        for fg in range(FPC // 128):
                            nc.tensor.matmul(
                                ps[:, _ts(eb, 512)],
                                lhsT=oT_sb[:, fg, _ts(tb, 128)],
                                rhs=wp_sb[:, fg, _ts(eb, 512)],
                                start=(fg == 0),
                                stop=(fg == FPC // 128 - 1),
                            )
                    o_sb = ob.tile([128, 1024], f32, tag="osb")
                    nc.scalar.copy(o_sb, ps)
                    nc.sync.dma_start(out=out.ap()[_ts(tb, 128), :], in_=o_sb)

    nc.compile()
    return nc


def _part_major(a, p=128):
    """[n*128, m] -> [128, n, m] with partition index innermost in rows."""
    n = a.shape[0] // p
    return np.ascontiguousarray(a.reshape(n, p, a.shape[1]).transpose(1, 0, 2))


def make_in_maps(x, W_attn, b_attn, W_proj):
    """Build the 8 per-core input maps (core = 2*b + g)."""
    x = np.asarray(x, dtype=np.float32)
    W_attn = np.asarray(W_attn, dtype=np.float32)
    b_attn = np.asarray(b_attn, dtype=np.float32)
    W_proj = np.asarray(W_proj, dtype=np.float32)

    # causal 0/1 masks for the 4 diagonal alignments (k-tile 128 vs q-block 512)
    kk = np.arange(128)[:, None]
    qq = np.arange(512)[None, :]
    msk = np.stack(
        [(qq >= j * 128 + kk) for j in range(4)], axis=1
    ).astype(BF16)  # [128, 4, 512]

    in_maps = []
    for b in range(B):
        xt = _part_major(np.ascontiguousarray(x[b].T)).astype(BF16)  # [128,8,S]
        for g in range(2):
            qs = W_attn[:, g * FPC : (g + 1) * FPC]
            ks = W_attn[:, E + g * FPC : E + (g + 1) * FPC]
            vs = W_attn[:, 2 * E + g * FPC : 2 * E + (g + 1) * FPC]
            wqk = _part_major(np.concatenate([qs, ks], axis=1)).astype(BF16)
            wv = _part_major(vs).astype(BF16)
            wp = _part_major(W_proj[g * FPC : (g + 1) * FPC, :]).astype(BF16)
            bq = b_attn[g * FPC : (g + 1) * FPC]
            bk = b_attn[E + g * FPC : E + (g + 1) * FPC]
            bqk = np.concatenate([bq, bk])[None, :].astype(BF16)
            bv = b_attn[2 * E + g * FPC : 2 * E + (g + 1) * FPC][None, :].astype(
                BF16
            )
            in_maps.append(
                {
                    "xt": xt,
                    "wqk": np.ascontiguousarray(wqk),
                    "wv": np.ascontiguousarray(wv),
                    "wp": np.ascontiguousarray(wp),
                    "bqk": np.ascontiguousarray(bqk),
                    "bv": np.ascontiguousarray(bv),
                    "msk": np.ascontiguousarray(msk),
                }
            )
    return in_maps


def get_program():
    if "nc" not in _cache:
        _cache["nc"] = _build_program()
    return _cache["nc"]


def gather(results, b_proj):
    b_proj = np.asarray(b_proj, dtype=np.float32)
    out = np.empty((B, S, E), dtype=np.float32)
    for b in range(B):
        out[b] = results[2 * b]["out"] + results[2 * b + 1]["out"] + b_proj
    return out


def kernel(x, W_attn, b_attn, W_proj, b_proj):
    nc = get_program()
    in_maps = make_in_maps(x, W_attn, b_attn, W_proj)
    res = bass_utils.run_bass_kernel_spmd(nc, in_maps, core_ids=list(range(NCORE)))
    return gather(res.results, b_proj)


# revision 16
# speedup vs baseline: 1.4807x; 1.2349x over previous
"""Trainium2 Bass kernel for causal multi-head attention (B=4, S=2048, E=1024, H=16).

Sharding: 8 cores = (batch b in 0..3) x (head-group g in 0..1); each core
computes one batch and 8 heads end-to-end:
  - column-parallel QKV projection (only its heads' columns)
  - causal attention for its 8 heads
  - row-parallel output projection -> partial [S, E]
The two partials per batch are summed on the host (plus b_proj). No on-device
collectives are needed.

Device dataflow (per core), all matmuls in bf16 with fp32 PSUM accumulation:
  - Q^T, K^T computed directly in [feature, token] layout (out = W^T @ x^T),
    so attention needs no transposes. K^T is pre-scaled by 1/sqrt(d)=0.125.
  - scores S^T[k,q] = (K^T tile).T @ Q^T, two k-tiles fused per [128,1024]
    PSUM tile; one ACT exp evacuates both; causal mask via multiplicative
    0/1 bf16 masks on diagonal tiles (fully-masked tiles skipped).
  - AV: lhsT = [V | ones]; PSUM rows 0..63 = O^T (unnormalized), row 64 =
    softmax denominator. 1/denom via ACT exp(-ln(d)) (DVE reciprocal is
    ~3.3us for a 1-partition row; ACT is ~0.9us), PE ones-matmul broadcast,
    one DVE multiply -> normalized O^T bf16. Normalize chains are deferred
    into the next (head, q-block) group so the PE never stalls on them.
  - All PSUM evacuation goes through ACT: DVE reads of PE-written PSUM race
    the matmul drain on HW (observed flaky garbage on first execution).
  - c_proj contracts the stacked O^T [512, S] against W_proj rows.
"""

import os
import sys

import numpy as np

for _p in ("/opt/trn_rl_repo", "/root/.axon_site/_ro/trn_rl_repo"):
    if os.path.isdir(_p) and _p not in sys.path:
        sys.path.append(_p)

import ml_dtypes  # noqa: E402

import concourse.bass as bass  # noqa: E402
import concourse.tile as tile  # noqa: E402
from concourse import bacc, bass_utils, hw_specs, mybir  # noqa: E402

# The act-table chooser assigns each activation the first table set containing
# its function, which ping-pongs Exp ("exp_and_others") and Ln ("natural_log")
# and inserts a 1.3us ACT_TABLE_LOAD per normalize chain (~65 per program).
# Restrict Exp/Ln to the combined "natural_log_exp_and_others" set (its
# act_func_set_id is preserved) so one load covers the whole kernel.
_orig_gat = hw_specs.get_activation_tables


def _shaped_gat(arch):
    t = _orig_gat(arch)
    if "natural_log_exp_and_others" in t:
        for name, funcs in t.items():
            if name != "natural_log_exp_and_others":
                funcs.discard(mybir.ActivationFunctionType.Exp)
                funcs.discard(mybir.ActivationFunctionType.Ln)
    return t


hw_specs.get_activation_tables = _shaped_gat
bacc.get_activation_tables = _shaped_gat

BF16 = ml_dtypes.bfloat16

B, S, E, H = 4, 2048, 1024, 16
D = E // H            # 64
NCORE = 8
HPC = H // 2          # heads per core = 8
KT = E // 128         # contraction tiles over E = 8
TB512 = S // 512      # 4
TB128 = S // 128      # 16
FPC = HPC * D         # features per core in attention output = 512

_cache: dict = {}


def _ts(i, n):
    return slice(i * n, (i + 1) * n)


def _build_program():
    bf = mybir.dt.bfloat16
    f32 = mybir.dt.float32
    nc = bacc.Bacc("TRN2", target_bir_lowering=False, debug=False)

    xt = nc.dram_tensor("xt", [128, KT, S], bf, kind="ExternalInput")
    wqk = nc.dram_tensor("wqk", [128, KT, 2 * FPC], bf, kind="ExternalInput")
    wv = nc.dram_tensor("wv", [128, KT, FPC], bf, kind="ExternalInput")
    wp = nc.dram_tensor("wp", [128, FPC // 128, E], bf, kind="ExternalInput")
    bqk = nc.dram_tensor("bqk", [1, 2 * FPC], bf, kind="ExternalInput")
    bv = nc.dram_tensor("bv", [1, FPC], bf, kind="ExternalInput")
    msk = nc.dram_tensor("msk", [128, 4, 512], bf, kind="ExternalInput")
    out = nc.dram_tensor("out", [S, E], f32, kind="ExternalOutput")
    # scratch for the softmax-reciprocal partition broadcast (DMA bounce)
    rscr = nc.dram_tensor("rscr", [HPC * TB512, 512], mybir.dt.float32)

    Exp = mybir.ActivationFunctionType.Exp
    Ln = mybir.ActivationFunctionType.Ln

    with tile.TileContext(nc) as tc:
        with (
            tc.tile_pool(name="big", bufs=1) as big,
            tc.tile_pool(name="pp", bufs=3) as pp,
            tc.tile_pool(name="sm", bufs=3) as sm,
            tc.tile_pool(name="ob", bufs=3) as ob,
        ):
            xts = [
                big.tile([128, S], bf, tag=f"xt{k}", name=f"xt{k}")
                for k in range(KT)
            ]
            wqks = [
                big.tile([128, 2 * FPC], bf, tag=f"wqk{k}", name=f"wqk{k}")
                for k in range(KT)
            ]
            wv_sb = big.tile([128, KT, FPC], bf, tag="wv")
            wp_sb = big.tile([128, FPC // 128, E], bf, tag="wp")
            bqk_sb = big.tile([1, 2 * FPC], bf, tag="bqk")
            bv_sb = big.tile([1, FPC], bf, tag="bv")
            msk_sb = big.tile([128, 4, 512], bf, tag="msk")
            ones_sb = big.tile([1, S], bf, tag="ones")
            # per-pair Q^T/K^T/O^T tiles so interleaved QKV writes don't
            # WAR-serialize against another pair's attention reads
            qTs = [big.tile([128, S], bf, tag=f"qT{p}", name=f"qT{p}") for p in range(4)]
            kTs = [big.tile([128, S], bf, tag=f"kT{p}", name=f"kT{p}") for p in range(4)]
            oTs = [big.tile([128, S], bf, tag=f"oT{p}", name=f"oT{p}") for p in range(4)]
            vone_sb = big.tile([128, TB128, HPC, D + 1], bf, tag="vone")

            nc.sync.dma_start(out=bqk_sb, in_=bqk.ap())
            for k in range(KT):
                nc.sync.dma_start(out=wqks[k], in_=wqk.ap()[:, k, :])
                nc.sync.dma_start(out=xts[k], in_=xt.ap()[:, k, :])
            nc.sync.dma_start(out=wv_sb, in_=wv.ap())
            nc.sync.dma_start(out=bv_sb, in_=bv.ap())
            nc.sync.dma_start(out=msk_sb, in_=msk.ap())
            nc.sync.dma_start(out=wp_sb, in_=wp.ap())

            nc.vector.memset(ones_sb, 1.0)
            nc.vector.memset(vone_sb[:, :, :, D : D + 1], 1.0)

            # Attention-phase PSUM pools are also used to double-buffer the
            # upfront QKV/V projection groups (same [128,1024] tile shape).
            with (
                tc.tile_pool(name="sp", bufs=2, space="PSUM") as ps_sp,
                tc.tile_pool(name="av", bufs=2, space="PSUM") as ps_av,
                tc.tile_pool(name="qk", bufs=1, space="PSUM") as ps_qk,
            ):

                def emit_qk_group(fb, tbp, pool, tag):
                    """One Q^T/K^T projection group: 18 matmuls + 1 cast.
                    Yields after each instruction so it can be interleaved."""
                    ps = pool.tile([128, 1024], f32, tag=tag, name=f"qkv_{fb}_{tbp}")
                    for half in range(2):
                        tb = 2 * tbp + half
                        hs = _ts(half, 512)
                        for kt in range(KT):
                            nc.tensor.matmul(
                                ps[:, hs],
                                lhsT=wqks[kt][:, _ts(fb, 128)],
                                rhs=xts[kt][:, _ts(tb, 512)],
                                start=(kt == 0),
                                stop=False,
                            )
                            yield
                        nc.tensor.matmul(
                            ps[:, hs],
                            lhsT=bqk_sb[0:1, _ts(fb, 128)],
                            rhs=ones_sb[0:1, _ts(tb, 512)],
                            start=False,
                            stop=True,
                        )
                        yield
                    if fb < 4:
                        nc.scalar.copy(qTs[fb][:, _ts(tbp, 1024)], ps)
                    else:
                        nc.scalar.mul(kTs[fb - 4][:, _ts(tbp, 1024)], ps, 0.125)
                    yield

                # ---- Phase 1a: pair-0 Q^T/K^T upfront (dense) ----
                pools = [ps_qk, ps_sp]
                gi = 0
                for fb in (0, 4):
                    for tbp in range(TB512 // 2):
                        pool = pools[gi % 2]
                        for _ in emit_qk_group(
                            fb, tbp, pool, "qkvi" if pool is ps_qk else "sp"
                        ):
                            pass
                        gi += 1

                # ---- Phase 1b: V = x @ Wv (+bias), [token, feature] ----
                for tbp in range(TB128 // 2):
                    pool = pools[tbp % 2]
                    ps = pool.tile(
                        [128, 1024], f32, tag="qkvi" if pool is ps_qk else "sp"
                    )
                    for half in range(2):
                        tb = 2 * tbp + half
                        hs = _ts(half, 512)
                        for kt in range(KT):
                            nc.tensor.matmul(
                                ps[:, hs],
                                lhsT=xts[kt][:, _ts(tb, 128)],
                                rhs=wv_sb[:, kt, :],
                                start=(kt == 0),
                                stop=False,
                            )
                        nc.tensor.matmul(
                            ps[:, hs],
                            lhsT=ones_sb[0:1, 0:128],
                            rhs=bv_sb[0:1, :],
                            start=False,
                            stop=True,
                        )
                    nc.scalar.copy(
                        vone_sb[:, _ts(tbp, 2), :, 0:D],
                        ps[:, :].rearrange("p (t h d) -> p t h d", t=2, d=D),
                    )

                # ---- QKV feed for pairs 1..3, interleaved into attention ----
                def qkv_feed():
                    for pr in range(1, 4):
                        for fb in (pr, pr + 4):
                            for tbp in range(TB512 // 2):
                                yield from (
                                    (pr, None)
                                    for _ in emit_qk_group(fb, tbp, ps_qk, "qkvi")
                                )

                feed = qkv_feed()
                feed_total = 3 * 2 * (TB512 // 2) * 19  # 228 emissions
                feed_state = {"done": 0, "cur_pr": 1}

                def pump(n):
                    for _ in range(n):
                        got = next(feed, None)
                        if got is None:
                            feed_state["cur_pr"] = 5
                            return
                        feed_state["done"] += 1
                        feed_state["cur_pr"] = got[0]

                def drain_feed_through(pr):
                    """Emit QKV work until every pair <= pr is complete.
                    Required before attention reads qTs[pr]/kTs[pr]: Tile only
                    sees RAW deps for writes emitted before the read."""
                    while feed_state["cur_pr"] <= pr:
                        pump(1)

                # ---- c_proj feed: interleaved into pair-3 attention as the
                # QKV feed runs dry there (its work must precede pair 3).
                # A tb group unlocks once every head has normalized its
                # q-block (pair 3 / h=7 is the last writer of oTs[3]).
                def cproj_group(tb):
                    ps = ps_qk.tile([128, 1024], f32, tag="qkvi", name=f"pj{tb}")
                    for eb in range(2):
                        for fg in range(FPC // 128):
                            nc.tensor.matmul(
                                ps[:, _ts(eb, 512)],
                                lhsT=oTs[fg][:, _ts(tb, 128)],
                                rhs=wp_sb[:, fg, _ts(eb, 512)],
                                start=(fg == 0),
                                stop=(fg == FPC // 128 - 1),
                            )
                            yield
                    o_sb = ob.tile([128, 1024], f32, tag="osb", name=f"ob{tb}")
                    nc.scalar.copy(o_sb, ps)
                    yield
                    nc.sync.dma_start(out=out.ap()[_ts(tb, 128), :], in_=o_sb)
                    yield

                def cproj_feed():
                    for tb in range(TB128):
                        while tb >= cp_state["unlocked"]:
                            yield False  # not allowed yet; no emission
                        yield from (True for _ in cproj_group(tb))

                cp_state = {"unlocked": 0}
                cfeed = cproj_feed()

                def pump_cproj(n):
                    for _ in range(n):
                        got = next(cfeed, None)
                        if got is None or got is False:
                            return

                # ---- Phase 2: causal attention per head ----
                deferred = []  # pending normalize chains (one per group)

                def normalize(av, pair, qoff, qb, idx):
                    t1 = sm.tile([1, 512], f32, tag="t1")
                    nc.scalar.activation(t1, av[D : D + 1, :], Ln)
                    r_sb = sm.tile([1, 512], f32, tag="r")
                    nc.scalar.activation(r_sb, t1, Exp, scale=-1.0)
                    o_all = sm.tile([D, 512], f32, tag="o")
                    nc.scalar.copy(o_all, av[0:D, :])
                    # broadcast 1/denom across partitions via a DRAM bounce
                    nc.sync.dma_start(out=rscr.ap()[idx : idx + 1, :], in_=r_sb)
                    bc_sb = sm.tile([D, 512], f32, tag="bcsb")
                    rap = rscr.ap()[idx : idx + 1, :]
                    bcast = bass.AP(
                        tensor=rap.tensor,
                        offset=rap.offset,
                        ap=[[0, D]] + list(rap.ap)[1:],
                    )
                    nc.sync.dma_start(out=bc_sb, in_=bcast)
                    nc.vector.tensor_mul(
                        oTs[pair][qoff : qoff + D, _ts(qb, 512)], o_all, bc_sb
                    )

                step_no = [0]
                for h in range(HPC):
                    pair = h // 2
                    qoff = (h % 2) * D
                    drain_feed_through(pair)
                    for qb in range(TB512):
                        npairs = 2 * qb + 2  # fused ki-pairs (4qb+4 k-tiles)
                        av = ps_av.tile([D + 1, 512], f32, tag="av")
                        pend = None

                        def do_av(p, p_sb, av=av, h=h, npairs=npairs):
                            for half in range(2):
                                ki = 2 * p + half
                                nc.tensor.matmul(
                                    av,
                                    lhsT=vone_sb[:, ki, h, :],
                                    rhs=p_sb[:, _ts(half, 512)],
                                    start=(ki == 0),
                                    stop=(ki == 2 * npairs - 1),
                                )

                        for p in range(npairs):
                            sp = ps_sp.tile([128, 1024], f32, tag="sp")
                            for half in range(2):
                                ki = 2 * p + half
                                nc.tensor.matmul(
                                    sp[:, _ts(half, 512)],
                                    lhsT=kTs[pair][qoff : qoff + D, _ts(ki, 128)],
                                    rhs=qTs[pair][qoff : qoff + D, _ts(qb, 512)],
                                    start=True,
                                    stop=True,
                                )
                            if pend is not None:
                                do_av(*pend)
                            # keep the PE dense during ACT-paced stretches
                            step_no[0] += 1
                            want = (step_no[0] * feed_total) // 160
                            pump(want - feed_state["done"])
                            p_sb = pp.tile([128, 1024], bf, tag="p")
                            nc.scalar.activation(p_sb, sp, Exp)
                            if p >= 2 * qb:  # both halves are diagonal tiles
                                j = 2 * (p - 2 * qb)
                                nc.vector.tensor_mul(
                                    p_sb,
                                    p_sb,
                                    msk_sb[:, j : j + 2, :].rearrange(
                                        "k j q -> k (j q)"
                                    ),
                                )
                            pend = (p, p_sb)
                            if p == 1:
                                while deferred:
                                    fn, dh, dqb = deferred.pop(0)
                                    fn()
                                    if dh == HPC - 1:
                                        cp_state["unlocked"] = 4 * (dqb + 1)
                            if pair == 3:
                                pump_cproj(2)
                        do_av(*pend)
                        deferred.append(
                            (
                                lambda av=av, pair=pair, qoff=qoff, qb=qb, idx=h
                                * TB512
                                + qb: normalize(av, pair, qoff, qb, idx),
                                h,
                                qb,
                            )
                        )
                pump(feed_total)  # drain any remaining QKV work
                while deferred:
                    fn, dh, dqb = deferred.pop(0)
                    fn()
                cp_state["unlocked"] = TB128
                while next(cfeed, None) is not None:
                    pass


    nc.compile()
    return nc


def _part_major(a, p=128):
    """[n*128, m] -> [128, n, m] with partition index innermost in rows."""
    n = a.shape[0] // p
    return np.ascontiguousarray(a.reshape(n, p, a.shape[1]).transpose(1, 0, 2))


def make_in_maps(x, W_attn, b_attn, W_proj):
    """Build the 8 per-core input maps (core = 2*b + g)."""
    x = np.asarray(x, dtype=np.float32)
    W_attn = np.asarray(W_attn, dtype=np.float32)
    b_attn = np.asarray(b_attn, dtype=np.float32)
    W_proj = np.asarray(W_proj, dtype=np.float32)

    # causal 0/1 masks for the 4 diagonal alignments (k-tile 128 vs q-block 512)
    kk = np.arange(128)[:, None]
    qq = np.arange(512)[None, :]
    msk = np.stack(
        [(qq >= j * 128 + kk) for j in range(4)], axis=1
    ).astype(BF16)  # [128, 4, 512]

    in_maps = []
    for b in range(B):
        xt = _part_major(np.ascontiguousarray(x[b].T)).astype(BF16)  # [128,8,S]
        for g in range(2):
            qs = W_attn[:, g * FPC : (g + 1) * FPC]
            ks = W_attn[:, E + g * FPC : E + (g + 1) * FPC]
            vs = W_attn[:, 2 * E + g * FPC : 2 * E + (g + 1) * FPC]
            wqk = _part_major(np.concatenate([qs, ks], axis=1)).astype(BF16)
            wv = _part_major(vs).astype(BF16)
            wp = _part_major(W_proj[g * FPC : (g + 1) * FPC, :]).astype(BF16)
            bq = b_attn[g * FPC : (g + 1) * FPC]
            bk = b_attn[E + g * FPC : E + (g + 1) * FPC]
            bqk = np.concatenate([bq, bk])[None, :].astype(BF16)
            bv = b_attn[2 * E + g * FPC : 2 * E + (g + 1) * FPC][None, :].astype(
                BF16
            )
            in_maps.append(
                {
                    "xt": xt,
                    "wqk": np.ascontiguousarray(wqk),
                    "wv": np.ascontiguousarray(wv),
                    "wp": np.ascontiguousarray(wp),
                    "bqk": np.ascontiguousarray(bqk),
                    "bv": np.ascontiguousarray(bv),
                    "msk": np.ascontiguousarray(msk),
                }
            )
    return in_maps


def get_program():
    if "nc" not in _cache:
        _cache["nc"] = _build_program()
    return _cache["nc"]


def gather(results, b_proj):
    b_proj = np.asarray(b_proj, dtype=np.float32)
    out = np.empty((B, S, E), dtype=np.float32)
    for b in range(B):
        out[b] = results[2 * b]["out"] + results[2 * b + 1]["out"] + b_proj
    return out


def kernel(x, W_attn, b_attn, W_proj, b_proj):
    nc = get_program()
    in_maps = make_in_maps(x, W_attn, b_attn, W_proj)
    res = bass_utils.run_bass_kernel_spmd(nc, in_maps, core_ids=list(range(NCORE)))
    return gather(res.results, b_proj)
